# revision 1
# baseline (speedup 1.0000x reference)
"""Trainium2 Bass kernel for an attention block (pre-LN attn + pre-LN SiLU MLP).

Sharding: data-parallel over batch b — one batch element per NeuronCore, 8 cores,
no collectives. Each core runs the full block on its [4096, 256] slice.

Per-core dataflow (T=4096 tokens, d=256, mlp=1024):
  x (token-major) --LN1--> xn --PE transpose--> xnT (feature-major, f32r)
  QT = Wq' @ xnT + rq ; KT = Wk' @ xnT + rk     (feature-major [d, T], bf16)
  V  = xnT.T @ WvT' + rv                        (token-major, bf16, [1,0] tail cols)
  S^T[j,i] = K @ Q^T  (PSUM) --exp(S/16) (ACT)--> AT (bf16)
  O'[i, :d+2] = sum_j AT[:,j,i].T @ V'[j]       (softmax row-sums arrive in col d)
  x2 = x + O'[:, :d] / O'[:, d]                 (in-place residual)
  x2 --LN2--> xn2 --transpose--> xn2T ; hT = silu(W1' @ xn2T + b1r)   (f32r)
  out = x2 + hT.T @ W2T + b2
gamma/beta are folded into the weights on the host (exact when gamma=1, beta=0).
Attention matmuls run in bf16 (softmax averaging cancels the rounding; also
enables separate LDWEIGHTS+FWL on the PE); projections/MLP run in float32r
(TF32-like). Measured end-to-end vs the fp32 reference: rel_l2 ~1.9e-4.
"""

import numpy as np

import concourse.bacc as bacc
import concourse.mybir as mybir
import concourse.tile as tile
from concourse.bass_utils import run_bass_kernel_spmd
from concourse.masks import make_identity

P = 128
D = 256
KD = D // P            # 2 d-chunks of 128
M = 1024
MO = M // P            # 8 mlp chunks of 128
EPS = 1e-5
SCALE = 1.0 / 16.0     # d ** -0.5

F32 = mybir.dt.float32
F32R = mybir.dt.float32r
AF = mybir.ActivationFunctionType
ALU = mybir.AluOpType

N_CORES = 8
FULL_T = 4096


def build(T=FULL_T, n_cores=N_CORES, silu_af=None, reps=1, loop_reps=None,
          attn_bf16=True, at_bufs=2, sps_bufs=2, ops_bufs=3, av_fp8=False):
    silu_af = silu_af or AF.Silu
    BF16 = mybir.dt.bfloat16
    F8 = mybir.dt.float8e4
    att_dt = BF16 if attn_bf16 else F32R
    av_dt = F8 if av_fp8 else att_dt
    VW = 272 if av_fp8 else D + 2   # V row width (fp8 pair-step must be 16B-aligned)
    NB = T // P            # token blocks
    JC = NB                # j-chunks (keys)
    IG = min(512 if attn_bf16 else 256, T)  # i-group width
    NIG = T // IG
    IB = IG // P           # i-blocks per group
    JCG = 2 if IG == 512 else 4  # j-chunks per PSUM/exp group (PSUM banks)
    TG = min(512, T)       # mlp token-group width
    NTG = T // TG
    TB = TG // P

    nc = bacc.Bacc("TRN2", target_bir_lowering=False, debug=False,
                   num_devices=n_cores)

    x_d = nc.dram_tensor("x", [T, D], F32, kind="ExternalInput")
    wqT_d = nc.dram_tensor("wqT", [D, D], F32, kind="ExternalInput")
    wkT_d = nc.dram_tensor("wkT", [D, D], F32, kind="ExternalInput")
    wvT_d = nc.dram_tensor("wvT", [D, D], F32, kind="ExternalInput")
    rq_d = nc.dram_tensor("rq", [D], F32, kind="ExternalInput")
    rk_d = nc.dram_tensor("rk", [D], F32, kind="ExternalInput")
    rv_d = nc.dram_tensor("rv", [D], F32, kind="ExternalInput")
    w1T_d = nc.dram_tensor("w1T", [D, M], F32, kind="ExternalInput")
    b1r_d = nc.dram_tensor("b1r", [M], F32, kind="ExternalInput")
    w2T_d = nc.dram_tensor("w2T", [M, D], F32, kind="ExternalInput")
    b2_d = nc.dram_tensor("b2", [D], F32, kind="ExternalInput")
    if av_fp8:
        onesv_d = nc.dram_tensor("onesv_f8", [VW - D], F8, kind="ExternalInput")
    else:
        onesv_d = nc.dram_tensor("onesv_bf" if attn_bf16 else "onesv", [2],
                                 mybir.dt.bfloat16 if attn_bf16 else F32,
                                 kind="ExternalInput")
    out_d = nc.dram_tensor("out", [T, D], F32, kind="ExternalOutput")

    import contextlib

    with tile.TileContext(nc) as tc:
      if loop_reps is not None:
          rep_iter = [0]
          loop_cm = tc.For_i(0, loop_reps, 1)
      else:
          rep_iter = range(reps)
          loop_cm = contextlib.nullcontext()
      with loop_cm:
       for _rep in rep_iter:
        with tc.tile_pool(name="glob", bufs=1) as glob:
            xsb = glob.tile([P, NB, D], F32)
            x_r = x_d.ap().rearrange("(p tt) d -> p tt d", p=P)
            c0 = 0
            for xq in (2, 2, 4, 8, NB):
                xq = min(xq, NB - c0)
                if xq <= 0:
                    break
                nc.sync.dma_start(xsb[:, c0:c0 + xq, :], x_r[:, c0:c0 + xq, :])
                c0 += xq
            b2b = glob.tile([P, D], F32)
            nc.sync.dma_start(b2b[:], b2_d.ap()[None, :].to_broadcast([P, D]))
            rvb = glob.tile([P, D], F32)
            nc.sync.dma_start(rvb[:], rv_d.ap()[None, :].to_broadcast([P, D]))
            rqs = glob.tile([P, KD], F32)
            nc.sync.dma_start(rqs[:], rq_d.ap().rearrange("(ko p) -> p ko", p=P))
            rks = glob.tile([P, KD], F32)
            nc.sync.dma_start(rks[:], rk_d.ap().rearrange("(ko p) -> p ko", p=P))
            b1rs = glob.tile([P, MO], F32)
            nc.sync.dma_start(b1rs[:], b1r_d.ap().rearrange("(mo p) -> p mo", p=P))
            ident = glob.tile([P, P], F32)
            make_identity(nc, ident)
            epst = glob.tile([P, 1], F32)
            nc.vector.memset(epst[:], EPS)
            mv2 = glob.tile([P, NB, 2], F32)
            xt_all = glob.tile([P, 6, D], F32)
            xn2T_all = glob.tile([P, 2, KD, TG], F32R)
            w1s = glob.tile([P, KD, M], F32R)
            nc.sync.dma_start(w1s[:], w1T_d.ap().rearrange(
                "(ko p) m -> p ko m", p=P).bitcast(F32R))
            w2s = glob.tile([P, MO, D], F32R)
            nc.sync.dma_start(w2s[:], w2T_d.ap().rearrange(
                "(mo p) m -> p mo m", p=P).bitcast(F32R))

            def ln_stats(mv_all, tmp_pool, chunk=8):
                """Per-block mean/var -> mv_all [P, NB, 2]; sqrt+reciprocal
                in chunks so early blocks' rstd is ready before later stats
                finish (consecutive Sqrt instrs share one ACT table load)."""
                for c0 in range(0, NB, chunk):
                    ce = min(c0 + chunk, NB)
                    for b in range(c0, ce):
                        stats = tmp_pool.tile([P, 6], F32, tag="stats")
                        nc.vector.bn_stats(stats[:], xsb[:, b, :])
                        nc.vector.bn_aggr(mv_all[:, b, :], stats[:])
                    nc.scalar.activation(mv_all[:, c0:ce, 1],
                                         mv_all[:, c0:ce, 1],
                                         AF.Sqrt, bias=epst[:], scale=1.0)
                    nc.vector.reciprocal(mv_all[:, c0:ce, 1],
                                         mv_all[:, c0:ce, 1])

            def ln_apply(dst, src, mv_all, b):
                nc.gpsimd.tensor_scalar(out=dst, in0=src,
                                        scalar1=mv_all[:, b, 0:1],
                                        scalar2=mv_all[:, b, 1:2],
                                        op0=ALU.subtract, op1=ALU.mult)

            # ---- span: tensors alive through phase A + attention ----
            with tc.tile_pool(name="span", bufs=1) as span:
                QT = span.tile([P, KD, T], att_dt)
                KT = span.tile([P, KD, T], att_dt)
                V = span.tile([P, NB, VW], av_dt)
                ones_src = onesv_d.ap()[None, None, :].to_broadcast(
                    [P, NB, VW - D])
                if not attn_bf16 and not av_fp8:
                    ones_src = ones_src.bitcast(F32R)
                nc.sync.dma_start(V[:, :, D:VW], ones_src)

                # ---- phase A: LN1, xnT, QKV projections ----
                with tc.tile_pool(name="pa", bufs=1) as pa, \
                     tc.tile_pool(name="pa_tmp", bufs=6) as pat, \
                     tc.tile_pool(name="pa_tr", bufs=2, space="PSUM") as paps, \
                     tc.tile_pool(name="pa_v", bufs=2, space="PSUM") as paps2, \
                     tc.tile_pool(name="pa_qk", bufs=2, space="PSUM") as paps3:
                    xnT = pa.tile([P, KD, T], F32R)
                    wqs = pa.tile([P, KD, D], F32R)
                    nc.sync.dma_start(wqs[:], wqT_d.ap().rearrange(
                        "(ko p) m -> p ko m", p=P).bitcast(F32R))
                    wks = pa.tile([P, KD, D], F32R)
                    nc.sync.dma_start(wks[:], wkT_d.ap().rearrange(
                        "(ko p) m -> p ko m", p=P).bitcast(F32R))
                    wvs = pa.tile([P, KD, D], F32R)
                    nc.sync.dma_start(wvs[:], wvT_d.ap().rearrange(
                        "(ko p) m -> p ko m", p=P).bitcast(F32R))

                    mv1 = pa.tile([P, NB, 2], F32)
                    LNC = 8
                    for b in range(NB):
                        if b % LNC == 0:
                            ce = min(b + LNC, NB)
                            for bs in range(b, ce):
                                stats = pat.tile([P, 6], F32, tag="stats")
                                nc.vector.bn_stats(stats[:], xsb[:, bs, :])
                                nc.vector.bn_aggr(mv1[:, bs, :], stats[:])
                            nc.scalar.activation(mv1[:, b:ce, 1],
                                                 mv1[:, b:ce, 1],
                                                 AF.Sqrt, bias=epst[:],
                                                 scale=1.0)
                            nc.vector.reciprocal(mv1[:, b:ce, 1],
                                                 mv1[:, b:ce, 1])
                        xt = pat.tile([P, D], F32, tag="xn")
                        ln_apply(xt[:], xsb[:, b, :], mv1, b)
                        tps = paps.tile([P, KD, P], F32, tag="tr")
                        for k in range(KD):
                            nc.tensor.transpose(tps[:, k, :],
                                                xt[:, k * P:(k + 1) * P],
                                                ident[:])
                        nc.vector.tensor_copy(xnT[:, :, b * P:(b + 1) * P],
                                              tps[:])
                    for b in range(NB):
                        vps = paps2.tile([P, D], F32, tag="v")
                        for k in range(KD):
                            nc.tensor.matmul(vps[:], xnT[:, k, b * P:(b + 1) * P],
                                             wvs[:, k, :],
                                             start=(k == 0), stop=(k == KD - 1))
                        nc.vector.tensor_tensor(out=V[:, b, 0:D], in0=vps[:],
                                                in1=rvb[:], op=ALU.add)

                    CC = min(512, T)
                    NCC = T // CC
                    for dst, wsb, rbias in ((QT, wqs, rqs), (KT, wks, rks)):
                        for k2 in range(KD):
                            for cc in range(NCC):
                                qps = paps3.tile([P, CC], F32, tag="qk")
                                for k in range(KD):
                                    nc.tensor.matmul(
                                        qps[:], wsb[:, k, k2 * P:(k2 + 1) * P],
                                        xnT[:, k, cc * CC:(cc + 1) * CC],
                                        start=(k == 0), stop=(k == KD - 1))
                                nc.vector.tensor_scalar_add(
                                    out=dst[:, k2, cc * CC:(cc + 1) * CC],
                                    in0=qps[:], scalar1=rbias[:, k2:k2 + 1])

                # ---- attention ----
                with tc.tile_pool(name="at_pool", bufs=at_bufs) as atp, \
                     tc.tile_pool(name="attn_tmp", bufs=3) as att, \
                     tc.tile_pool(name="s_ps", bufs=sps_bufs, space="PSUM") as sps, \
                     tc.tile_pool(name="o_ps", bufs=ops_bufs, space="PSUM") as ops_:
                    AT = None
                    for ig in range(NIG):
                        if AT is None or at_bufs > 1:
                            AT = atp.tile([P, JC, IG], av_dt, tag="AT")
                        for jg in range(JC // JCG):
                            sp = sps.tile([P, JCG, IG], F32, tag="s")
                            for jl in range(JCG):
                                jc = jg * JCG + jl
                                for k in range(KD):
                                    nc.tensor.matmul(
                                        sp[:, jl, :],
                                        KT[:, k, jc * P:(jc + 1) * P],
                                        QT[:, k, ig * IG:(ig + 1) * IG],
                                        start=(k == 0), stop=(k == KD - 1))
                            nc.scalar.activation(
                                AT[:, jg * JCG:(jg + 1) * JCG, :], sp[:],
                                AF.Exp, scale=SCALE)
                        for ib in range(IB):
                            bb = ig * IB + ib
                            op_ = ops_.tile([P, D + 2], F32, tag="o")
                            if av_fp8:
                                for jp in range(JC // 2):
                                    nc.tensor.matmul(
                                        op_[:],
                                        AT[:, 2 * jp:2 * jp + 2,
                                           ib * P:(ib + 1) * P],
                                        V[:, 2 * jp:2 * jp + 2, 0:D + 2],
                                        start=(jp == 0),
                                        stop=(jp == JC // 2 - 1),
                                        perf_mode=mybir.MatmulPerfMode.DoubleRow)
                            else:
                                for jc in range(JC):
                                    nc.tensor.matmul(
                                        op_[:], AT[:, jc, ib * P:(ib + 1) * P],
                                        V[:, jc, 0:D + 2],
                                        start=(jc == 0), stop=(jc == JC - 1))
                            rec = att.tile([P, 1], F32, tag="rec")
                            nc.vector.reciprocal(rec[:], op_[:, D:D + 1])
                            osb = att.tile([P, D], F32, tag="osb")
                            nc.vector.tensor_scalar_mul(out=osb[:],
                                                        in0=op_[:, 0:D],
                                                        scalar1=rec[:])
                            nc.gpsimd.tensor_add(out=xsb[:, bb, :],
                                                 in0=xsb[:, bb, :], in1=osb[:])
                            stats2 = att.tile([P, 6], F32, tag="stats2")
                            nc.vector.bn_stats(stats2[:], xsb[:, bb, :])
                            nc.vector.bn_aggr(mv2[:, bb, :], stats2[:])

            # ---- MLP ----
            with tc.tile_pool(name="mlp_w", bufs=1) as mp, \
                 tc.tile_pool(name="mlp_db", bufs=2) as mdb, \
                 tc.tile_pool(name="mlp_tmp", bufs=3) as mt, \
                 tc.tile_pool(name="m_tr", bufs=2, space="PSUM") as mps, \
                 tc.tile_pool(name="m_h", bufs=2, space="PSUM") as hps, \
                 tc.tile_pool(name="m_y", bufs=2, space="PSUM") as yps:
                out_r = out_d.ap().rearrange("(p tt) d -> p tt d", p=P)
                nc.scalar.activation(mv2[:, :, 1], mv2[:, :, 1],
                                     AF.Sqrt, bias=epst[:], scale=1.0)
                nc.vector.reciprocal(mv2[:, :, 1], mv2[:, :, 1])
                for tg in range(NTG):
                    xn2T = xn2T_all[:, tg % 2]
                    for bloc in range(TB):
                        bb = tg * TB + bloc
                        xt = xt_all[:, bb % 6, :]
                        ln_apply(xt[:], xsb[:, bb, :], mv2, bb)
                        tps = mps.tile([P, KD, P], F32, tag="tr2")
                        for k in range(KD):
                            nc.tensor.transpose(tps[:, k, :],
                                                xt[:, k * P:(k + 1) * P],
                                                ident[:])
                        nc.vector.tensor_copy(
                            xn2T[:, :, bloc * P:(bloc + 1) * P], tps[:])
                    hT = mdb.tile([P, MO, TG], F32R, tag="hT")
                    for mo in range(MO):
                        hp = hps.tile([P, TG], F32, tag="h")
                        for k in range(KD):
                            nc.tensor.matmul(hp[:], w1s[:, k, mo * P:(mo + 1) * P],
                                             xn2T[:, k, :],
                                             start=(k == 0), stop=(k == KD - 1))
                        nc.scalar.activation(hT[:, mo, :], hp[:], silu_af,
                                             bias=b1rs[:, mo:mo + 1], scale=1.0)
                    for bloc in range(TB):
                        bb = tg * TB + bloc
                        yp = yps.tile([P, D], F32, tag="y")
                        for mo in range(MO):
                            nc.tensor.matmul(yp[:],
                                             hT[:, mo, bloc * P:(bloc + 1) * P],
                                             w2s[:, mo, :],
                                             start=(mo == 0), stop=(mo == MO - 1))
                        ot = mt.tile([P, D], F32, tag="ot")
                        nc.vector.tensor_add(out=ot[:], in0=yp[:],
                                             in1=xsb[:, bb, :])
                        nc.gpsimd.tensor_add(out=ot[:], in0=ot[:], in1=b2b[:])
                        nc.sync.dma_start(out_r[:, bb, :], ot[:])

    nc.compile()
    return nc


def prepare_inputs(x, w_qkv, gamma1, beta1, gamma2, beta2, w1, b1, w2, b2):
    """Host-side prep: slice w_qkv, fold gamma/beta into weights, transpose."""
    f8 = np.float64
    x = np.asarray(x, np.float32)
    B = x.shape[0]
    T = x.shape[1] * x.shape[2]
    w_qkv = np.asarray(w_qkv, f8)
    g1 = np.asarray(gamma1, f8)
    be1 = np.asarray(beta1, f8)
    g2 = np.asarray(gamma2, f8)
    be2 = np.asarray(beta2, f8)
    w1 = np.asarray(w1, f8)
    w2 = np.asarray(w2, f8)
    wq, wk, wv = w_qkv[0::3], w_qkv[1::3], w_qkv[2::3]
    f32c = lambda a: np.ascontiguousarray(a, np.float32)
    common = {
        "wqT": f32c((wq * g1[None, :]).T),
        "wkT": f32c((wk * g1[None, :]).T),
        "wvT": f32c((wv * g1[None, :]).T),
        "rq": f32c(wq @ be1),
        "rk": f32c(wk @ be1),
        "rv": f32c(wv @ be1),
        "w1T": f32c((w1 * g2[None, :]).T),
        "b1r": f32c(np.asarray(b1, f8) + w1 @ be2),
        "w2T": f32c(w2.T),
        "b2": f32c(b2),
        "onesv": np.array([1.0, 0.0], np.float32),
    }
    import ml_dtypes
    common["onesv_bf"] = np.array([1.0, 0.0], ml_dtypes.bfloat16)
    common["onesv_f8"] = np.array([1.0] + [0.0] * 15,
                                   ml_dtypes.float8_e4m3)
    xf = x.reshape(B, T, x.shape[3])
    in_maps = [dict(common, x=np.ascontiguousarray(xf[c])) for c in range(B)]
    return in_maps


_CACHE = {}


def get_nc():
    if "nc" not in _CACHE:
        _CACHE["nc"] = build()
    return _CACHE["nc"]


def kernel(x, w_qkv, gamma1, beta1, gamma2, beta2, w1, b1, w2, b2):
    x = np.asarray(x, np.float32)
    B, N, H, Dd = x.shape
    assert (B, N, H, Dd) == (8, 1024, 4, 256), x.shape
    in_maps = prepare_inputs(x, w_qkv, gamma1, beta1, gamma2, beta2,
                             w1, b1, w2, b2)
    nc = get_nc()
    res = run_bass_kernel_spmd(nc, in_maps, core_ids=list(range(N_CORES)))
    out = np.stack([res.results[c]["out"] for c in range(B)], 0)
    return np.ascontiguousarray(out.reshape(B, N, H, Dd).astype(np.float32))



# revision 14
# speedup vs baseline: 1.8975x; 1.8975x over previous
"""Trainium2 Bass kernel for an attention block (pre-LN attn + pre-LN SiLU MLP).

Sharding: data-parallel over batch b — one batch element per NeuronCore, 8 cores,
no collectives. Each core runs the full block on its [4096, 256] slice.

v3 design (per core, T=4096 tokens, d=256, mlp=1024):
  x (token-major f32) --LN1--> xt (fp8 e4m3) --PE transpose--> PSUM --DMA-->
      xnT (feature-major fp8, zero engine cost for the move)
  Q/K/V projections: fp8 DoubleRow single-shot matmuls against 16x-scaled fp8
      weights -> f32 PSUM -> quantize(+bias) to fp8 QT8/KT8 (ACT Identity) and
      V8 (DVE add) ; all values carry a 16x scale, compensated downstream
  S' = K8 @ Q8^T (= 256*S) via ONE fp8 DoubleRow matmul per j-chunk (PSUM f32)
  A8 = exp(S'/4096) as fp8e4m3, split between two engines:
      ACT: true exp (f8 out); DVE: Schraudolph bit-trick
      u8 = u8(a*S' + b) bitcast=> e4m3 (exp2 linearization, +-4% per weight,
      mean error cancels in the softmax normalization)
  O[i, :258] = A8 @ V8 fp8 DoubleRow, 16 pair-matmuls; row sums of A8 land in
      col 256 via a 16.0-constant tail column in V8 (16x matches V's scale)
  x2 = x + O[:, :256] / O[:, 256]   (ACT Copy with per-row scale, Pool add)
  x2 --LN2--> bf16 --transpose--> PSUM --DMA--> xn2T
  hT = silu(W1 @ xn2T + b1) (bf16) ; out = x2 + hT.T @ W2T + b2
MLP matmuls run in bf16; attention in fp8e4m3 DoubleRow (0.5 cyc/col, both
128-contraction chunks fused per instruction). gamma/beta fold into weights on
the host. Measured rel_l2 vs the fp32 reference: ~4e-3 (gate: 2e-2).
"""

import numpy as np

import concourse.bacc as bacc
import concourse.mybir as mybir
import concourse.tile as tile
from concourse.bass_utils import run_bass_kernel_spmd
from concourse.masks import make_identity

P = 128
D = 256
KD = D // P            # 2 d-chunks of 128
M = 1024
MO = M // P            # 8 mlp chunks of 128
EPS = 1e-5
WS = 16.0              # fp8 weight prescale (q,k,v all carry 16x)
SCALE = 1.0 / 16.0     # d ** -0.5
SCALE_S = SCALE / (WS * WS)             # logits = S' * SCALE_S
A8 = SCALE_S * 8.0 / float(np.log(2.0))  # S' -> e4m3 exponent-code scale
B8 = 56.0 + 0.5 - 0.344                 # 7*8 bias + trunc-centering - ln-corr

F32 = mybir.dt.float32
BF16 = mybir.dt.bfloat16
F8 = mybir.dt.float8e4
U8 = mybir.dt.uint8
AF = mybir.ActivationFunctionType
ALU = mybir.AluOpType

N_CORES = 8
FULL_T = 4096


def build(T=FULL_T, n_cores=N_CORES, silu_af=None, reps=1, loop_reps=None,
          at_bufs=2, sps_bufs=3, ops_bufs=2, osb_eng="act", qkq_eng="mix",
          v_eng="dve", mlp_major="mo", conv_engines=("act", "dve")):
    silu_af = silu_af or AF.Silu
    VW = 272               # V row width; fp8 pair-step must be 16B-aligned
    NB = T // P            # token blocks
    JC = NB                # j-chunks (keys)
    JCG = 2                # j-chunks per PSUM/exp-convert unit
    IG = min(512, T)       # i-group width
    NIG = T // IG
    IB = IG // P           # i-blocks per group
    TG = min(512, T)       # mlp token-group width
    NTG = T // TG
    TB = TG // P

    nc = bacc.Bacc("TRN2", target_bir_lowering=False, debug=False,
                   num_devices=n_cores)

    x_d = nc.dram_tensor("x", [T, D], F32, kind="ExternalInput")
    wqT_d = nc.dram_tensor("wqT8", [D, D], F8, kind="ExternalInput")
    wkT_d = nc.dram_tensor("wkT8", [D, D], F8, kind="ExternalInput")
    wvT_d = nc.dram_tensor("wvT8", [D, D], F8, kind="ExternalInput")
    rq_d = nc.dram_tensor("rq16", [D], F32, kind="ExternalInput")
    rk_d = nc.dram_tensor("rk16", [D], F32, kind="ExternalInput")
    rv_d = nc.dram_tensor("rv16", [D], F32, kind="ExternalInput")
    w1T_d = nc.dram_tensor("w1T", [D, M], BF16, kind="ExternalInput")
    b1r_d = nc.dram_tensor("b1r", [M], F32, kind="ExternalInput")
    w2T_d = nc.dram_tensor("w2T", [M, D], BF16, kind="ExternalInput")
    b2_d = nc.dram_tensor("b2", [D], F32, kind="ExternalInput")
    onesv_d = nc.dram_tensor("onesv16_f8", [VW - D], F8, kind="ExternalInput")
    out_d = nc.dram_tensor("out", [T, D], F32, kind="ExternalOutput")

    import contextlib

    with tile.TileContext(nc) as tc:
      if loop_reps is not None:
          rep_iter = [0]
          loop_cm = tc.For_i(0, loop_reps, 1)
      else:
          rep_iter = range(reps)
          loop_cm = contextlib.nullcontext()
      with loop_cm:
       for _rep in rep_iter:
        with tc.tile_pool(name="glob", bufs=1) as glob:
            xsb = glob.tile([P, NB, D], F32)
            x_r = x_d.ap().rearrange("(p tt) d -> p tt d", p=P)
            c0 = 0
            for xq in (2, 2, 4, 8, NB):
                xq = min(xq, NB - c0)
                if xq <= 0:
                    break
                nc.sync.dma_start(xsb[:, c0:c0 + xq, :], x_r[:, c0:c0 + xq, :])
                c0 += xq
            b2b = glob.tile([P, D], F32)
            nc.sync.dma_start(b2b[:], b2_d.ap()[None, :].to_broadcast([P, D]))
            rvb = glob.tile([P, D], F32)
            nc.sync.dma_start(rvb[:], rv_d.ap()[None, :].to_broadcast([P, D]))
            rqs = glob.tile([P, KD], F32)
            nc.sync.dma_start(rqs[:], rq_d.ap().rearrange("(ko p) -> p ko", p=P))
            rks = glob.tile([P, KD], F32)
            nc.sync.dma_start(rks[:], rk_d.ap().rearrange("(ko p) -> p ko", p=P))
            b1rs = glob.tile([P, MO], F32)
            nc.sync.dma_start(b1rs[:], b1r_d.ap().rearrange("(mo p) -> p mo", p=P))
            identb = glob.tile([P, P], BF16)
            make_identity(nc, identb)
            epst = glob.tile([P, 1], F32)
            nc.vector.memset(epst[:], EPS)
            mv2 = glob.tile([P, NB, 2], F32)
            xt_all = glob.tile([P, 6, D], BF16)
            xn2T_all = glob.tile([P, 2, KD, TG], BF16)
            w1s = glob.tile([P, KD, M], BF16)
            nc.sync.dma_start(w1s[:], w1T_d.ap().rearrange(
                "(ko p) m -> p ko m", p=P))
            w2s = glob.tile([P, MO, D], BF16)
            nc.sync.dma_start(w2s[:], w2T_d.ap().rearrange(
                "(mo p) m -> p mo m", p=P))

            def ln_apply(dst, src, mv_all, b, eng=None):
                eng = eng or nc.gpsimd
                eng.tensor_scalar(out=dst, in0=src,
                                  scalar1=mv_all[:, b, 0:1],
                                  scalar2=mv_all[:, b, 1:2],
                                  op0=ALU.subtract, op1=ALU.mult)

            # ---- span: tensors alive through phase A + attention ----
            with tc.tile_pool(name="span", bufs=1) as span:
                QT = span.tile([P, KD, T], F8)
                KT = span.tile([P, KD, T], F8)
                V = span.tile([P, NB, VW], F8)
                nc.sync.dma_start(
                    V[:, :, D:VW],
                    onesv_d.ap()[None, None, :].to_broadcast([P, NB, VW - D]))

                # ---- phase A: LN1, xnT, QKV projections (fp8) ----
                with tc.tile_pool(name="pa", bufs=1) as pa, \
                     tc.tile_pool(name="pa_tmp", bufs=6) as pat, \
                     tc.tile_pool(name="pa_tr", bufs=3, space="PSUM") as paps, \
                     tc.tile_pool(name="pa_v", bufs=2, space="PSUM") as paps2, \
                     tc.tile_pool(name="pa_qk", bufs=2, space="PSUM") as paps3:
                    xnT = pa.tile([P, KD, T], F8)
                    wqs = pa.tile([P, KD, D], F8)
                    nc.sync.dma_start(wqs[:], wqT_d.ap().rearrange(
                        "(ko p) m -> p ko m", p=P))
                    wks = pa.tile([P, KD, D], F8)
                    nc.sync.dma_start(wks[:], wkT_d.ap().rearrange(
                        "(ko p) m -> p ko m", p=P))
                    wvs = pa.tile([P, KD, D], F8)
                    nc.sync.dma_start(wvs[:], wvT_d.ap().rearrange(
                        "(ko p) m -> p ko m", p=P))

                    mv1 = pa.tile([P, NB, 2], F32)
                    qki = 0
                    LNC = 8
                    for b in range(NB):
                        if b % LNC == 0:
                            ce = min(b + LNC, NB)
                            for bs in range(b, ce):
                                stats = pat.tile([P, 6], F32, tag="stats")
                                nc.vector.bn_stats(stats[:], xsb[:, bs, :])
                                nc.vector.bn_aggr(mv1[:, bs, :], stats[:])
                            nc.scalar.activation(mv1[:, b:ce, 1],
                                                 mv1[:, b:ce, 1],
                                                 AF.Sqrt, bias=epst[:],
                                                 scale=1.0)
                            nc.vector.reciprocal(mv1[:, b:ce, 1],
                                                 mv1[:, b:ce, 1])
                        xt = pat.tile([P, D], BF16, tag="xn")
                        ln_apply(xt[:], xsb[:, b, :], mv1, b)
                        tps = paps.tile([P, KD, P], BF16, tag="tr")
                        for k in range(KD):
                            nc.tensor.transpose(tps[:, k, :],
                                                xt[:, k * P:(k + 1) * P],
                                                identb[:])
                        if b % 2 == 0:
                            nc.vector.tensor_copy(
                                xnT[:, :, b * P:(b + 1) * P], tps[:])
                        else:
                            nc.scalar.activation(
                                xnT[:, :, b * P:(b + 1) * P], tps[:],
                                AF.Copy)
                        vps = paps2.tile([P, D], F32, tag="v")
                        nc.tensor.matmul(vps[:], xnT[:, :, b * P:(b + 1) * P],
                                         wvs[:], start=True, stop=True,
                                         perf_mode=mybir.MatmulPerfMode.DoubleRow)
                        if v_eng == "act":
                            # requires rv == 0 (Copy takes no tensor bias)
                            nc.scalar.activation(V[:, b, 0:D], vps[:], AF.Copy)
                        else:
                            nc.vector.tensor_tensor(out=V[:, b, 0:D],
                                                    in0=vps[:],
                                                    in1=rvb[:], op=ALU.add)
                        if b % 4 == 3:
                            cc = b // 4
                            CC = 4 * P
                            for dst, wsb, rbias in ((KT, wks, rks),
                                                    (QT, wqs, rqs)):
                                for k2 in range(KD):
                                    qps = paps3.tile([P, CC], F32, tag="qk")
                                    nc.tensor.matmul(
                                        qps[:], wsb[:, :, k2 * P:(k2 + 1) * P],
                                        xnT[:, :, cc * CC:(cc + 1) * CC],
                                        start=True, stop=True,
                                        perf_mode=mybir.MatmulPerfMode.DoubleRow)
                                    use_act = (qkq_eng == "act" or
                                               (qkq_eng == "mix" and
                                                qki % 2 == 0))
                                    qki += 1
                                    if use_act:
                                        nc.scalar.activation(
                                            dst[:, k2, cc * CC:(cc + 1) * CC],
                                            qps[:], AF.Identity,
                                            bias=rbias[:, k2:k2 + 1],
                                            scale=1.0)
                                    else:
                                        nc.vector.tensor_scalar_add(
                                            out=dst[:, k2,
                                                    cc * CC:(cc + 1) * CC],
                                            in0=qps[:],
                                            scalar1=rbias[:, k2:k2 + 1])

                # ---- attention ----
                conv_i = 0
                with tc.tile_pool(name="at_pool", bufs=at_bufs) as atp, \
                     tc.tile_pool(name="attn_tmp", bufs=3) as att, \
                     tc.tile_pool(name="s_ps", bufs=sps_bufs, space="PSUM") as sps, \
                     tc.tile_pool(name="o_ps", bufs=ops_bufs, space="PSUM") as ops_:
                    AT = None
                    for ig in range(NIG):
                        if AT is None or at_bufs > 1:
                            AT = atp.tile([P, JC, IG], F8, tag="AT")
                        for jg in range(JC // JCG):
                            sp = sps.tile([P, JCG, IG], F32, tag="s")
                            for jl in range(JCG):
                                jc = jg * JCG + jl
                                nc.tensor.matmul(
                                    sp[:, jl, :],
                                    KT[:, :, jc * P:(jc + 1) * P],
                                    QT[:, :, ig * IG:(ig + 1) * IG],
                                    start=True, stop=True,
                                    perf_mode=mybir.MatmulPerfMode.DoubleRow)
                            eng = conv_engines[conv_i % len(conv_engines)]
                            conv_i += 1
                            j0 = jg * JCG
                            if eng == "act":
                                nc.scalar.activation(
                                    AT[:, j0:j0 + JCG, :], sp[:],
                                    AF.Exp, scale=SCALE_S)
                            else:
                                nc.vector.tensor_scalar(
                                    out=AT[:, j0:j0 + JCG, :].bitcast(U8),
                                    in0=sp[:],
                                    scalar1=float(A8), scalar2=float(B8),
                                    op0=ALU.mult, op1=ALU.add)
                        for ib in range(IB):
                            bb = ig * IB + ib
                            op_ = ops_.tile([P, D + 2], F32, tag="o")
                            for jp in range(JC // 2):
                                nc.tensor.matmul(
                                    op_[:],
                                    AT[:, 2 * jp:2 * jp + 2,
                                       ib * P:(ib + 1) * P],
                                    V[:, 2 * jp:2 * jp + 2, 0:D + 2],
                                    start=(jp == 0),
                                    stop=(jp == JC // 2 - 1),
                                    perf_mode=mybir.MatmulPerfMode.DoubleRow)
                            rec = att.tile([P, 1], F32, tag="rec")
                            nc.vector.reciprocal(rec[:], op_[:, D:D + 1])
                            osb = att.tile([P, D], F32, tag="osb")
                            if osb_eng == "act":
                                nc.scalar.activation(osb[:], op_[:, 0:D],
                                                     AF.Copy, bias=0.0,
                                                     scale=rec[:])
                            else:
                                nc.vector.tensor_scalar_mul(out=osb[:],
                                                            in0=op_[:, 0:D],
                                                            scalar1=rec[:])
                            nc.gpsimd.tensor_add(out=xsb[:, bb, :],
                                                 in0=xsb[:, bb, :], in1=osb[:])
                            stats2 = att.tile([P, 6], F32, tag="stats2")
                            nc.vector.bn_stats(stats2[:], xsb[:, bb, :])
                            nc.vector.bn_aggr(mv2[:, bb, :], stats2[:])

            # ---- MLP ----
            with tc.tile_pool(name="mlp_db", bufs=2) as mdb, \
                 tc.tile_pool(name="mlp_tmp", bufs=3) as mt, \
                 tc.tile_pool(name="m_tr", bufs=2, space="PSUM") as mps, \
                 tc.tile_pool(name="m_h", bufs=2, space="PSUM") as hps, \
                 tc.tile_pool(name="m_y", bufs=1, space="PSUM") as yps:
                out_r = out_d.ap().rearrange("(p tt) d -> p tt d", p=P)
                nc.scalar.activation(mv2[:, :, 1], mv2[:, :, 1],
                                     AF.Sqrt, bias=epst[:], scale=1.0)
                nc.vector.reciprocal(mv2[:, :, 1], mv2[:, :, 1])
                for tg in range(NTG):
                    xn2T = xn2T_all[:, tg % 2]
                    for bloc in range(TB):
                        bb = tg * TB + bloc
                        xt = xt_all[:, bb % 6, :]
                        ln_apply(xt[:], xsb[:, bb, :], mv2, bb)
                        tps = mps.tile([P, KD, P], BF16, tag="tr2")
                        for k in range(KD):
                            nc.tensor.transpose(tps[:, k, :],
                                                xt[:, k * P:(k + 1) * P],
                                                identb[:])
                        nc.vector.tensor_copy(
                            xn2T[:, :, bloc * P:(bloc + 1) * P], tps[:])
                    hT = mdb.tile([P, MO, TG], BF16, tag="hT")
                    if mlp_major == "mo":
                        yp = [yps.tile([P, D], F32, tag=f"y{bloc}",
                                       name=f"yp{bloc}")
                              for bloc in range(TB)]
                        for mo in range(MO):
                            hp = hps.tile([P, TG], F32, tag="h")
                            for k in range(KD):
                                nc.tensor.matmul(
                                    hp[:], w1s[:, k, mo * P:(mo + 1) * P],
                                    xn2T[:, k, :],
                                    start=(k == 0), stop=(k == KD - 1))
                            nc.scalar.activation(hT[:, mo, :], hp[:], silu_af,
                                                 bias=b1rs[:, mo:mo + 1],
                                                 scale=1.0)
                            for bloc in range(TB):
                                nc.tensor.matmul(
                                    yp[bloc][:],
                                    hT[:, mo, bloc * P:(bloc + 1) * P],
                                    w2s[:, mo, :],
                                    start=(mo == 0), stop=(mo == MO - 1))
                        for bloc in range(TB):
                            bb = tg * TB + bloc
                            ot = mt.tile([P, D], F32, tag="ot")
                            nc.vector.tensor_add(out=ot[:], in0=yp[bloc][:],
                                                 in1=xsb[:, bb, :])
                            nc.gpsimd.tensor_add(out=ot[:], in0=ot[:],
                                                 in1=b2b[:])
                            nc.sync.dma_start(out_r[:, bb, :], ot[:])
                    else:
                        for mo in range(MO):
                            hp = hps.tile([P, TG], F32, tag="h")
                            for k in range(KD):
                                nc.tensor.matmul(
                                    hp[:], w1s[:, k, mo * P:(mo + 1) * P],
                                    xn2T[:, k, :],
                                    start=(k == 0), stop=(k == KD - 1))
                            nc.scalar.activation(hT[:, mo, :], hp[:], silu_af,
                                                 bias=b1rs[:, mo:mo + 1],
                                                 scale=1.0)
                        for bloc in range(TB):
                            bb = tg * TB + bloc
                            yp0 = yps.tile([P, D], F32, tag="y0")
                            for mo in range(MO):
                                nc.tensor.matmul(
                                    yp0[:],
                                    hT[:, mo, bloc * P:(bloc + 1) * P],
                                    w2s[:, mo, :],
                                    start=(mo == 0), stop=(mo == MO - 1))
                            ot = mt.tile([P, D], F32, tag="ot")
                            nc.vector.tensor_add(out=ot[:], in0=yp0[:],
                                                 in1=xsb[:, bb, :])
                            nc.gpsimd.tensor_add(out=ot[:], in0=ot[:],
                                                 in1=b2b[:])
                            nc.sync.dma_start(out_r[:, bb, :], ot[:])

    nc.compile()
    return nc


def prepare_inputs(x, w_qkv, gamma1, beta1, gamma2, beta2, w1, b1, w2, b2):
    """Host-side prep: slice w_qkv, fold gamma/beta into weights, transpose.
    Q/K/V weights are prescaled by WS=16 and cast to fp8e4m3; the 16x (and
    16x16 in the logits) is compensated by SCALE_S and the 16.0 ones column.
    """
    import ml_dtypes
    f8d = np.float64
    BF = ml_dtypes.bfloat16
    E4 = ml_dtypes.float8_e4m3
    x = np.asarray(x, np.float32)
    B = x.shape[0]
    T = x.shape[1] * x.shape[2]
    w_qkv = np.asarray(w_qkv, f8d)
    g1 = np.asarray(gamma1, f8d)
    be1 = np.asarray(beta1, f8d)
    g2 = np.asarray(gamma2, f8d)
    be2 = np.asarray(beta2, f8d)
    w1 = np.asarray(w1, f8d)
    w2 = np.asarray(w2, f8d)
    wq, wk, wv = w_qkv[0::3], w_qkv[1::3], w_qkv[2::3]
    f32c = lambda a: np.ascontiguousarray(a, np.float32)
    bfc = lambda a: np.ascontiguousarray(np.asarray(a, np.float32), BF)
    f8c = lambda a: np.ascontiguousarray(np.asarray(a, np.float32), E4)
    common = {
        "wqT8": f8c((wq * g1[None, :] * WS).T),
        "wkT8": f8c((wk * g1[None, :] * WS).T),
        "wvT8": f8c((wv * g1[None, :] * WS).T),
        "rq16": f32c(wq @ be1 * WS),
        "rk16": f32c(wk @ be1 * WS),
        "rv16": f32c(wv @ be1 * WS),
        "w1T": bfc((w1 * g2[None, :]).T),
        "b1r": f32c(np.asarray(b1, f8d) + w1 @ be2),
        "w2T": bfc(w2.T),
        "b2": f32c(b2),
        "onesv16_f8": np.array([WS] + [0.0] * 15, E4),
    }
    xf = x.reshape(B, T, x.shape[3])
    in_maps = [dict(common, x=np.ascontiguousarray(xf[c])) for c in range(B)]
    return in_maps


_CACHE = {}


def get_nc():
    if "nc" not in _CACHE:
        _CACHE["nc"] = build()
    return _CACHE["nc"]


def kernel(x, w_qkv, gamma1, beta1, gamma2, beta2, w1, b1, w2, b2):
    x = np.asarray(x, np.float32)
    B, N, H, Dd = x.shape
    assert (B, N, H, Dd) == (8, 1024, 4, 256), x.shape
    in_maps = prepare_inputs(x, w_qkv, gamma1, beta1, gamma2, beta2,
                             w1, b1, w2, b2)
    nc = get_nc()
    res = run_bass_kernel_spmd(nc, in_maps, core_ids=list(range(N_CORES)))
    out = np.stack([res.results[c]["out"] for c in range(B)], 0)
    return np.ascontiguousarray(out.reshape(B, N, H, Dd).astype(np.float32))
